# revision 1
# baseline (speedup 1.0000x reference)
"""Trainium2 Bass kernel for nn_CondenseSFR (BN+ReLU+shuffle+grouped1x1conv+reindex).

Algebra: out = einsum('nchw,cd->ndhw', conv(shuffle(relu(bn(x)))), index).
Everything except the ReLU is linear in the channel dimension, and the BN
scale inv = gamma*rsqrt(var+eps) is strictly positive, so
    relu(inv*x + b) = inv * relu(x + b/inv)
and the shuffle + grouped conv + reindex fold into a single dense 512x512
channel matrix applied after the ReLU:
    out[n,d,s] = sum_c B[d,c] * relu(x[n,c,s] + bprime[c])
with B = (index^T @ A) * inv[None,:],  A the shuffle-permuted block-diagonal
conv weight, bprime = (beta - mean*inv)/inv.

Device work per core (4 of 32 images, data-parallel over 8 cores), laid out
so the kernel stays at the ~410 GB/s HBM roofline wall-to-wall:
  - x image loads as 4 partition-tiles [128, 1025]; the extra column carries
    the per-channel ReLU bias so each ReLU has exactly one DMA dependency
    (TRN2 instructions support 1 hardware wait; a 2nd costs a standalone
    event-semaphore that can block the engine FIFO)
  - a single DMA queue sustains only ~250 GB/s, so input chunks alternate
    the two HWDGE rings (Sync+Scalar, all triggers pre-issued) and output
    stores split between the SWDGE queue and the HWDGE rings as they drain
  - VectorE: relu(x + bias_c) as one fused tensor_scalar(add,max), output
    rounded to fp32r (required by the fp32r matmul's producer check)
  - TensorE: ct-major PSUM accumulation - 8 banks hold all (d-tile, half)
    outputs of one image, so matmuls start when the first channel tile
    lands instead of after the last; fp32r runs 1 cycle/row (plain fp32 is 4)
  - ScalarE: PSUM -> SBUF evacuation (FD=1024, two banks per copy) between
    DMA triggers; last image's drain splits across ScalarE and VectorE
"""

import numpy as np

import concourse.bacc as bacc
import concourse.mybir as mybir
from concourse.tile import TileContext
from concourse.bass_utils import run_bass_kernel_spmd

EPS = 1e-5
GROUPS = 4
N, C, H, W = 32, 512, 32, 32
HW = H * W                 # 1024
HWB = HW + 1               # +1 bias column per channel tile
NCORES = 8
NPER = N // NCORES         # 4 images per core
CT = C // 128              # 4 channel tiles
F32 = mybir.dt.float32
F32R = mybir.dt.float32r

_NC_CACHE = None


def _build_nc():
    """Build the (SPMD, per-core) Bass program. Same program on all 8 cores."""
    nc = bacc.Bacc(None, enable_partition_id=False)

    x_d = nc.dram_tensor("x", [NPER, CT, 128, HWB], F32, kind="ExternalInput")
    w_d = nc.dram_tensor("w", [128, CT * CT * 128], F32R, kind="ExternalInput")
    o_d = nc.dram_tensor("o", [NPER, CT, 128, HW], F32, kind="ExternalOutput")

    with TileContext(nc) as tc:
        with (
            tc.tile_pool(name="const", bufs=1) as const,
            tc.tile_pool(name="xin", bufs=4) as xin,
            tc.tile_pool(name="act", bufs=3) as actp,
            tc.tile_pool(name="pp", bufs=8, space="PSUM") as pp,
            tc.tile_pool(name="outp", bufs=2) as outp,
        ):
            # Weight DMA first on the Scalar HWDGE ring. (Splitting it to
            # start matmuls earlier was tried and collapses the input-stream
            # rate to ~290 GB/s - keep it as one transfer.)
            wt = const.tile([128, CT * CT * 128], F32R)
            nc.scalar.dma_start(wt[:], w_d[:])

            # PE warm-up: the HAM clock gate holds the PE at 1.2 GHz until
            # ~3.4us of sustained activity. The PE is idle 7-14.5us while
            # the DMA ramp runs, so burn that window on dummy matmuls over
            # zeroed scratch - the real matmul stream then starts at 2.4 GHz.
            # The dummy PSUM tile shares tag ps0 and is released before the
            # first image needs the bank.
            wu = const.tile([128, 256], F32R)
            nc.vector.memset(wu[:].bitcast(F32), 0.0)
            wu_ps = pp.tile([128, 1024], F32, name="wu_ps", tag="ps0", bufs=1)
            for _ in range(30):
                nc.tensor.matmul(
                    wu_ps[:, :256], wu[:, :128], wu[:, :256],
                    start=True, stop=True,
                )

            # A single DMA queue sustains only ~250 GB/s; the ~410 GB/s HBM
            # cap needs >=2 active queues per direction. Inputs alternate the
            # two HWDGE rings (Sync + Scalar), all pre-issued; outputs split
            # between the SWDGE queue and the HWDGE rings once input drains.
            xts = []
            for n in range(NPER):
                xt = xin.tile([128, CT * HWB], F32, name=f"xt{n}", tag="xt")
                xts.append(xt)
                for ct in range(CT):
                    eng = nc.sync if ct % 2 == 0 else nc.scalar
                    eng.dma_start(xt[:, ct * HWB:(ct + 1) * HWB], x_d[n, ct])

            for n in range(NPER):
                xt = xts[n]
                ut = actp.tile([128, CT * HW], F32R)
                # 8 PSUM banks accumulate ct-major, so matmuls start as soon
                # as the first channel tile lands instead of after the last.
                # One [128,1024] (2-bank) tile per d-tile: each matmul writes
                # a single bank, but evacuation runs as one FD=1024 copy.
                pss = [
                    pp.tile([128, 1024], F32, name=f"ps_{n}_{j}", tag=f"ps{j}", bufs=1)
                    for j in range(CT)
                ]
                for ct in range(CT):
                    # relu(x + b) on DVE (fp32 tensor_scalar runs 2x there,
                    # and keeps ScalarE free to feed its HWDGE DMA ring)
                    nc.vector.tensor_scalar(
                        ut[:, ct * HW:(ct + 1) * HW],
                        xt[:, ct * HWB:ct * HWB + HW],
                        xt[:, ct * HWB + HW:(ct + 1) * HWB],
                        0.0,
                        mybir.AluOpType.add,
                        mybir.AluOpType.max,
                    )
                    for dt_ in range(CT):
                        for half in range(2):
                            wcol = (ct * CT + dt_) * 128
                            ucol = ct * HW + half * 512
                            nc.tensor.matmul(
                                pss[dt_][:, half * 512:(half + 1) * 512],
                                wt[:, wcol:wcol + 128],
                                ut[:, ucol:ucol + 512],
                                start=(ct == 0),
                                stop=(ct == CT - 1),
                            )

                last = n == NPER - 1
                ot = outp.tile([128, CT * HW], F32)
                for dt_ in range(CT):
                    ocol = dt_ * HW
                    # PSUM evacuation on ACT (DVE is busy with relus and
                    # would delay the out-stream start behind them); the
                    # last image's drain splits across ACT and DVE
                    if last and dt_ % 2 == 1:
                        nc.vector.tensor_copy(ot[:, ocol:ocol + HW], pss[dt_][:])
                    else:
                        nc.scalar.copy(ot[:, ocol:ocol + HW], pss[dt_][:])
                    # stores: dt1/dt3 on the Scalar HWDGE ring (input bytes
                    # there drain by ~30us), dt0/dt2 on SWDGE; the last
                    # image also uses the idle Sync ring
                    osl = ot[:, dt_ * HW:(dt_ + 1) * HW]
                    if dt_ % 2 == 1:
                        nc.scalar.dma_start(o_d[n, dt_], osl)
                    elif last:
                        nc.sync.dma_start(o_d[n, dt_], osl)
                    else:
                        nc.gpsimd.dma_start(o_d[n, dt_], osl)

    nc.finalize()
    return nc


def _prep_inputs(x, gamma, beta, running_mean, running_var, weight, index):
    """Fold BN/shuffle/conv/index into (per-core x shards, weight matrix)."""
    f64 = np.float64
    x = np.asarray(x)
    gamma = np.asarray(gamma)
    beta = np.asarray(beta)
    running_mean = np.asarray(running_mean)
    running_var = np.asarray(running_var)
    weight = np.asarray(weight)
    index = np.asarray(index)
    gamma = gamma.astype(f64)
    beta = beta.astype(f64)
    mean = running_mean.astype(f64)
    var = running_var.astype(f64)
    Wc = weight.reshape(C, C // GROUPS).astype(f64)   # (Cout, Cin_per_group)
    idx = index.astype(f64)

    inv = gamma / np.sqrt(var + EPS)                  # > 0
    beta_term = beta - mean * inv
    inv_safe = np.where(inv != 0.0, inv, 1.0)
    bprime = np.where(inv != 0.0, beta_term / inv_safe, 0.0)

    # A[o, c]: conv-after-shuffle as one 512x512 matrix.
    # shuffled channel g*128 + i comes from original channel i*GROUPS + g.
    A = np.zeros((C, C), dtype=f64)
    o = np.arange(C)
    i = np.arange(C // GROUPS)
    src = i[None, :] * GROUPS + (o[:, None] // (C // GROUPS))  # (512, 128)
    A[o[:, None], src] = Wc

    # out[d] = sum_c B[d,c] relu(x_c + bprime_c);  B = (idx^T @ A) * inv
    # Stationary operand is B^T[c, d] = (A^T @ idx) * inv[:, None]
    BT = (A.T @ idx) * inv[:, None]                   # (c, d)

    w_host = np.ascontiguousarray(
        BT.reshape(CT, 128, CT, 128).transpose(1, 0, 2, 3).reshape(128, CT * CT * 128)
    ).astype(np.float32)

    # x shards with the bias appended as column HW of each [128, HW] block
    xr = x.reshape(N, CT, 128, HW)
    bias_col = np.broadcast_to(
        bprime.astype(np.float32).reshape(CT, 128, 1), (N, CT, 128, 1)
    )
    xaug = np.concatenate([xr, bias_col], axis=3)      # (N, CT, 128, HWB)
    xaug = np.ascontiguousarray(
        xaug.reshape(NCORES, NPER, CT, 128, HWB), dtype=np.float32
    )
    return [{"x": xaug[k], "w": w_host} for k in range(NCORES)]


def _run(inputs, trace=False):
    global _NC_CACHE
    if _NC_CACHE is None:
        _NC_CACHE = _build_nc()
    in_maps = _prep_inputs(**inputs)
    res = run_bass_kernel_spmd(_NC_CACHE, in_maps, list(range(NCORES)), trace=trace)
    out = np.concatenate([res.results[k]["o"] for k in range(NCORES)], axis=0)
    out = out.reshape(N, C, H, W).astype(np.float32)
    return out, res


def kernel(**inputs):
    out, _ = _run(inputs, trace=False)
    return out



# revision 10
# speedup vs baseline: 1.3807x; 1.3807x over previous
"""Trainium2 Bass kernel for nn_CondenseSFR (BN+ReLU+shuffle+grouped1x1conv+reindex).

Algebra: out = einsum('nchw,cd->ndhw', conv(shuffle(relu(bn(x)))), index).
Everything except the ReLU is linear in the channel dimension, and the BN
scale inv = gamma*rsqrt(var+eps) is strictly positive, so
    relu(inv*x + b) = inv * relu(x + b/inv)
and the shuffle + grouped conv + reindex fold into a single dense 512x512
channel matrix applied after the ReLU:
    out[n,d,s] = sum_c B[d,c] * relu(x[n,c,s] + bprime[c])
with B = (index^T @ A) * inv[None,:],  A the shuffle-permuted block-diagonal
conv weight, bprime = (beta - mean*inv)/inv.

Tolerance is 2e-2 so the whole pipeline runs in bf16 (measured end-to-end
rel err ~3.3e-3): x, bias, weights and the stored output are bf16; matmul
accumulation stays fp32 in PSUM. vs the fp32 version this halves HBM
traffic (17.8 -> 8.9 MB/core, the fp32 kernel sat at the ~358 GB/s
HBM-per-core wall) and makes the PE the critical path (~27.5us of N=512
bf16 matmul streaming per core).

Device work per core (4 of 32 images, data-parallel over 8 cores):
  - x image loads as 4 partition-tiles [128, 1024] bf16; the per-channel
    ReLU bias rides in a separate tiny fp32 [128, CT] tensor loaded once
    up front (tensor_scalar's vector operand must be f32)
  - input chunks alternate the two HWDGE rings (Sync+Scalar); outputs
    split between the SWDGE queue and the HWDGE rings as they drain
  - VectorE: relu(x + bias_c) as one fused tensor_scalar(add,max) in
    bf16 (4x packed mode)
  - TensorE: ct-major PSUM accumulation - 8 banks hold all (d-tile, half)
    outputs of one image, so matmuls start when the first channel tile
    lands; bf16 weights get the automatic FWL fast weight load
  - PSUM -> SBUF evacuation casts fp32 -> bf16 (ACT engine, DVE helps on
    the last image) so stores are half-width too
"""

import numpy as np

import concourse.bacc as bacc
import concourse.mybir as mybir
from concourse.tile import TileContext
from concourse.bass_utils import run_bass_kernel_spmd

EPS = 1e-5
GROUPS = 4
N, C, H, W = 32, 512, 32, 32
HW = H * W                 # 1024
NCORES = 8
NPER = N // NCORES         # 4 images per core
CT = C // 128              # 4 channel tiles
F32 = mybir.dt.float32
BF16 = mybir.dt.bfloat16

_NC_CACHE = None


def _build_nc():
    """Build the (SPMD, per-core) Bass program. Same program on all 8 cores."""
    nc = bacc.Bacc(None, enable_partition_id=False)

    x_d = nc.dram_tensor("x", [NPER, CT, 128, HW], BF16, kind="ExternalInput")
    w_d = nc.dram_tensor("w", [128, CT * CT * 128], BF16, kind="ExternalInput")
    b_d = nc.dram_tensor("b", [128, CT], F32, kind="ExternalInput")
    o_d = nc.dram_tensor("o", [NPER, CT, 128, HW], BF16, kind="ExternalOutput")

    with TileContext(nc) as tc:
        with (
            tc.tile_pool(name="const", bufs=1) as const,
            tc.tile_pool(name="xin", bufs=4) as xin,
            tc.tile_pool(name="act", bufs=3) as actp,
            tc.tile_pool(name="pp", bufs=8, space="PSUM") as pp,
            tc.tile_pool(name="outp", bufs=2) as outp,
        ):
            # Bias (2KB, fp32 - tensor_scalar's vector operand must be f32)
            # then weights, both on the Scalar HWDGE ring.
            bt = const.tile([128, CT], F32)
            nc.scalar.dma_start(bt[:], b_d[:])
            wt = const.tile([128, CT * CT * 128], BF16)
            nc.scalar.dma_start(wt[:], w_d[:])

            # PE warm-up: the HAM clock gate holds the PE at 1.2 GHz until
            # ~3.4us of sustained activity. Burn the DMA-ramp window on
            # dummy matmuls over zeroed scratch so the real matmul stream
            # starts at 2.4 GHz. The dummy PSUM tile shares tag ps0 and is
            # released before the first image needs the bank.
            wu = const.tile([128, 256], BF16)
            nc.vector.memset(wu[:], 0.0)
            wu_ps = pp.tile([128, 1024], F32, name="wu_ps", tag="ps0", bufs=1)
            for _ in range(30):
                nc.tensor.matmul(
                    wu_ps[:, :256], wu[:, :128], wu[:, :256],
                    start=True, stop=True,
                )

            # Input chunks alternate the two HWDGE rings (Sync + Scalar),
            # all pre-issued; outputs split between the SWDGE queue and the
            # HWDGE rings once input drains.
            xts = []
            for n in range(NPER):
                xt = xin.tile([128, CT * HW], BF16, name=f"xt{n}", tag="xt")
                xts.append(xt)
                for ct in range(CT):
                    eng = nc.sync if ct % 2 == 0 else nc.scalar
                    eng.dma_start(xt[:, ct * HW:(ct + 1) * HW], x_d[n, ct])

            for n in range(NPER):
                xt = xts[n]
                ut = actp.tile([128, CT * HW], BF16)
                # 8 PSUM banks accumulate ct-major, so matmuls start as soon
                # as the first channel tile lands instead of after the last.
                # One [128,1024] (2-bank) tile per d-tile: each matmul writes
                # a single bank, but evacuation runs as one FD=1024 copy.
                pss = [
                    pp.tile([128, 1024], F32, name=f"ps_{n}_{j}", tag=f"ps{j}", bufs=1)
                    for j in range(CT)
                ]
                for ct in range(CT):
                    # relu(x + b) on DVE: bf16 tensor_scalar runs the 4x
                    # packed mode, and keeps ScalarE free for its HWDGE ring
                    nc.vector.tensor_scalar(
                        ut[:, ct * HW:(ct + 1) * HW],
                        xt[:, ct * HW:(ct + 1) * HW],
                        bt[:, ct:ct + 1],
                        0.0,
                        mybir.AluOpType.add,
                        mybir.AluOpType.max,
                    )
                    for dt_ in range(CT):
                        for half in range(2):
                            wcol = (ct * CT + dt_) * 128
                            ucol = ct * HW + half * 512
                            nc.tensor.matmul(
                                pss[dt_][:, half * 512:(half + 1) * 512],
                                wt[:, wcol:wcol + 128],
                                ut[:, ucol:ucol + 512],
                                start=(ct == 0),
                                stop=(ct == CT - 1),
                            )

                last = n == NPER - 1
                ot = outp.tile([128, CT * HW], BF16)
                for dt_ in range(CT):
                    ocol = dt_ * HW
                    # PSUM evacuation (with the fp32->bf16 cast) on ACT; the
                    # last image's drain splits across ACT and DVE
                    if last and dt_ % 2 == 1:
                        nc.vector.tensor_copy(ot[:, ocol:ocol + HW], pss[dt_][:])
                    else:
                        nc.scalar.copy(ot[:, ocol:ocol + HW], pss[dt_][:])
                    # stores: dt1/dt3 on the Scalar HWDGE ring (input bytes
                    # there drain early), dt0/dt2 on SWDGE; the last image
                    # also uses the idle Sync ring
                    osl = ot[:, dt_ * HW:(dt_ + 1) * HW]
                    if dt_ % 2 == 1:
                        nc.scalar.dma_start(o_d[n, dt_], osl)
                    elif last:
                        nc.sync.dma_start(o_d[n, dt_], osl)
                    else:
                        nc.gpsimd.dma_start(o_d[n, dt_], osl)

    nc.finalize()
    return nc


def _prep_inputs(x, gamma, beta, running_mean, running_var, weight, index):
    """Fold BN/shuffle/conv/index into (per-core x shards, weight matrix)."""
    f64 = np.float64
    x = np.asarray(x)
    gamma = np.asarray(gamma).astype(f64)
    beta = np.asarray(beta).astype(f64)
    mean = np.asarray(running_mean).astype(f64)
    var = np.asarray(running_var).astype(f64)
    weight = np.asarray(weight)
    index = np.asarray(index)
    Wc = weight.reshape(C, C // GROUPS).astype(f64)   # (Cout, Cin_per_group)
    idx = index.astype(f64)

    inv = gamma / np.sqrt(var + EPS)                  # > 0
    beta_term = beta - mean * inv
    inv_safe = np.where(inv != 0.0, inv, 1.0)
    bprime = np.where(inv != 0.0, beta_term / inv_safe, 0.0)

    # A[o, c]: conv-after-shuffle as one 512x512 matrix.
    # shuffled channel g*128 + i comes from original channel i*GROUPS + g.
    A = np.zeros((C, C), dtype=f64)
    o = np.arange(C)
    i = np.arange(C // GROUPS)
    src = i[None, :] * GROUPS + (o[:, None] // (C // GROUPS))  # (512, 128)
    A[o[:, None], src] = Wc

    # out[d] = sum_c B[d,c] relu(x_c + bprime_c);  B = (idx^T @ A) * inv
    # Stationary operand is B^T[c, d] = (A^T @ idx) * inv[:, None]
    BT = (A.T @ idx) * inv[:, None]                   # (c, d)

    bf16 = np.dtype(mybir.dt.np(BF16))

    w_host = np.ascontiguousarray(
        BT.reshape(CT, 128, CT, 128).transpose(1, 0, 2, 3).reshape(128, CT * CT * 128)
    ).astype(np.float32).astype(bf16)

    xr = np.ascontiguousarray(
        x.reshape(NCORES, NPER, CT, 128, HW)
    ).astype(bf16)
    b_host = np.ascontiguousarray(
        bprime.astype(np.float32).reshape(CT, 128).T
    )                                                  # (128, CT)
    return [{"x": xr[k], "w": w_host, "b": b_host} for k in range(NCORES)]


def _unpack_output(res):
    out = np.concatenate(
        [np.asarray(res.results[k]["o"]) for k in range(NCORES)], axis=0
    )
    return out.astype(np.float32).reshape(N, C, H, W)


def _run(inputs, trace=False):
    global _NC_CACHE
    if _NC_CACHE is None:
        _NC_CACHE = _build_nc()
    in_maps = _prep_inputs(**inputs)
    res = run_bass_kernel_spmd(_NC_CACHE, in_maps, list(range(NCORES)), trace=trace)
    return _unpack_output(res), res


def kernel(**inputs):
    out, _ = _run(inputs, trace=False)
    return out
